# revision 43
# baseline (speedup 1.0000x reference)
"""Single-head causal attention on 8 TRN2 NeuronCores.

Problem: x [8, 2048, 1024] f32, Wq/Wk/Wv [1024, 64] f32.
  q = x @ Wq ; k = x @ Wk ; v = x @ Wv        (per batch)
  out = softmax(causal(q k^T / 8)) @ v        [8, 2048, 64]

Sharding: data-parallel over batch — core i handles batch element i.
No collectives needed.

Per-core kernel (bf16 compute, f32 accumulate), per 512-token chunk c:
  1. DMA x chunk [128, 4, 1024] f32 -> SBUF (all 4 loads issued up
     front so DMA streams), cast to bf16 on GPSIMD (keeps DVE free).
  2. One DMA-xbar transpose per t-tile builds x^T [128 d-part, dc, t].
  3. Projections: lhsT = packed [Wq|Wk] per d-chunk -> Q^T,K^T [64, 512];
     lhsT = Wv -> V^T [64, 512]; V^T xbar-transposed (on the ACT DMA
     queue) to V [t-part, 4, 64] augmented with a ones column
     (softmax denominator for free).
  4. Scores in transposed orientation S^T[tk, tq] = K^T_slice.T @ Q^T
     (operands already h-on-partitions; no P transposes anywhere).
  5. exp on ACT (scale=1/8); causal diagonal via a 0/1 upper-triangular
     multiplicative bf16 mask.
  6. PV: out_aug^T[65, tq] += V_aug[ki].T @ P^T accumulated over k-tiles
     in PSUM; row 64 accumulates the softmax denominators.
  7. PE-transpose out_aug^T back to [tq, 65] (f32), scale rows by the
     reciprocal denominator, one DMA store per chunk.
"""

import numpy as np

import concourse.bass as bass
import concourse.tile as tile
from concourse import bacc, mybir
from concourse.bass_utils import run_bass_kernel_spmd

B, T, D, H = 8, 2048, 1024, 64
P = 128            # partitions / tile edge
ND = D // P        # 8 d-chunks
NT = T // P        # 16 token tiles
CW = 512           # chunk width (1 PSUM bank of f32)
NC = T // CW       # 4 chunks
KPC = CW // P      # 4 k-tiles per chunk

FP32 = mybir.dt.float32
BF16 = mybir.dt.bfloat16

_compiled = None
DEBUG_DUMP = False


def _build():
    nc = bacc.Bacc("TRN2", target_bir_lowering=False, debug=False, num_devices=8)

    x_d = nc.dram_tensor("x", [T, D], FP32, kind="ExternalInput").ap()
    wq_d = nc.dram_tensor("Wq", [D, H], FP32, kind="ExternalInput").ap()
    wk_d = nc.dram_tensor("Wk", [D, H], FP32, kind="ExternalInput").ap()
    wv_d = nc.dram_tensor("Wv", [D, H], FP32, kind="ExternalInput").ap()
    out_d = nc.dram_tensor("out", [T, H], FP32, kind="ExternalOutput").ap()
    dbg = {}
    if DEBUG_DUMP:
        dbg["xt0"] = nc.dram_tensor("xt0", [P, ND, CW], FP32,
                                    kind="ExternalOutput").ap()
        dbg["vaug0"] = nc.dram_tensor("vaug0", [P, KPC, H + 1], FP32,
                                      kind="ExternalOutput").ap()
        dbg["qt0"] = nc.dram_tensor("qt0", [H, CW], FP32,
                                    kind="ExternalOutput").ap()
        dbg["kt0"] = nc.dram_tensor("kt0", [H, CW], FP32,
                                    kind="ExternalOutput").ap()

    with tile.TileContext(nc) as tc:
        _kernel(tc, out_d, x_d, wq_d, wk_d, wv_d, dbg)

    nc.compile()
    return nc


def _kernel(tc, out_d, x_d, wq_d, wk_d, wv_d, dbg=None):
    nc = tc.nc
    from contextlib import ExitStack

    ctx = ExitStack()
    with ctx:
        const = ctx.enter_context(tc.tile_pool(name="const", bufs=1))
        wstage = ctx.enter_context(tc.tile_pool(name="wstage", bufs=2))
        xload = ctx.enter_context(tc.tile_pool(name="xload", bufs=4))
        xbf = ctx.enter_context(tc.tile_pool(name="xbf", bufs=6))
        xtp = ctx.enter_context(tc.tile_pool(name="xtp", bufs=1))
        qkv = ctx.enter_context(tc.tile_pool(name="qkv", bufs=1))
        vsb = ctx.enter_context(tc.tile_pool(name="vsb", bufs=1))
        ptp = ctx.enter_context(tc.tile_pool(name="ptp", bufs=6))
        otp = ctx.enter_context(tc.tile_pool(name="otp", bufs=2))
        osb = ctx.enter_context(tc.tile_pool(name="osb", bufs=4))
        small = ctx.enter_context(tc.tile_pool(name="small", bufs=4))
        pwork = ctx.enter_context(tc.tile_pool(name="pwork", bufs=3, space="PSUM"))
        pout = ctx.enter_context(tc.tile_pool(name="pout", bufs=1, space="PSUM"))

        # ---- constants ----
        # Packed projection weights per d-chunk: [Wq | Wk] -> [128, dc, 128]
        w_qk = const.tile([P, ND, P], BF16)
        w_v = const.tile([P, ND, H], BF16)

        def load_weights():
            for w_dram, dst in ((wq_d, w_qk[:, :, 0:H]),
                                (wk_d, w_qk[:, :, H:P]),
                                (wv_d, w_v[:, :, :])):
                stg = wstage.tile([P, ND, H], FP32, tag="wstage",
                                  name=f"stg_{w_dram.tensor.name}")
                nc.gpsimd.dma_start(
                    out=stg[:],
                    in_=w_dram.rearrange("(dc p) h -> p dc h", p=P))
                nc.gpsimd.tensor_copy(out=dst, in_=stg[:])

        # f32 identity for the PE output transpose
        ident = const.tile([P, P], FP32)
        from concourse.masks import make_identity
        make_identity(nc, ident[:])
        ident_bf = const.tile([P, P], BF16)
        make_identity(nc, ident_bf[:])

        # 0/1 upper-triangular (incl. diagonal) bf16 mask in [tk, tq]
        # orientation: valid when tq >= tk  (col >= row).
        tri01 = const.tile([P, P], BF16)
        nc.gpsimd.memset(tri01[:], 1.0)
        nc.gpsimd.affine_select(
            out=tri01[:], in_=tri01[:],
            compare_op=mybir.AluOpType.is_ge,
            fill=0.0, base=0,
            pattern=[[1, P]], channel_multiplier=-1)

        # V_aug per chunk: [128 t-part, 4 k-tiles, 80] with col 64 = 1.0.
        # The k-tile stride is padded 65 -> 80 elements so each xbar
        # transpose writes at a 32-byte-aligned SBUF offset (the ucode
        # DMA-transpose silently corrupts on misaligned outputs).
        VA = 80
        v_aug = []
        for c in range(NC):
            va = vsb.tile([P, KPC, VA], BF16, tag=f"vaug{c}", name=f"vaug{c}")
            nc.gpsimd.memset(va[:, :, H:H + 1], 1.0)
            v_aug.append(va)

        # ---- x: per-chunk load-group -> cast -> xbar transpose zipper ----
        # Loads for chunk c and the transposes for chunk c alternate on the
        # SP queue so the DMA engines stream densely and chunk 0's x^T is
        # ready early.
        xt_chunks = [xtp.tile([P, ND, CW], BF16, tag=f"xT{c}", name=f"xT{c}")
                     for c in range(NC)]

        x_r = x_d.rearrange("(c a p) d -> c p a d", p=P, a=KPC)

        xfs = {}

        def load_x(c):
            xf = xload.tile([P, KPC, D], FP32, tag="xf", name=f"xf{c}")
            nc.sync.dma_start(out=xf[:], in_=x_r[c])
            xfs[c] = xf

        def cast_transpose(c):
            cast_eng = nc.gpsimd if c == 2 else nc.vector
            for a in range(KPC):
                xb = xbf.tile([P, D], BF16, tag="xb", name=f"xb{c}_{a}")
                cast_eng.tensor_copy(out=xb[:], in_=xfs[c][:, a, :])
                if c == NC - 1:
                    # last chunk: transpose on the (idle) PE instead of the
                    # backlogged DMA xbar
                    ps_x = pwork.tile([P, ND, P], BF16, tag="pwork",
                                      name=f"ps_x{c}_{a}")
                    for dc in range(ND):
                        nc.tensor.transpose(ps_x[:, dc, :],
                                            xb[:, dc * P:(dc + 1) * P],
                                            ident_bf[:])
                    nc.vector.tensor_copy(
                        out=xt_chunks[c][:, :, a * P:(a + 1) * P],
                        in_=ps_x[:])
                else:
                    nc.sync.dma_start(
                        out=xt_chunks[c][:, :, a * P:(a + 1) * P],
                        in_=xb[:],
                        transpose=True)

        load_weights()
        load_x(0)
        cast_transpose(0)
        load_x(1)
        cast_transpose(1)
        load_x(2)
        load_x(3)

        # ---- per-chunk: projections, attention, output ----
        qt_chunks, kt_chunks = [], []
        stores = []
        out_stage = []
        for c in range(NC):
            xt = xt_chunks[c]
            ps_qk = pwork.tile([P, CW], FP32, tag="pwork")
            for dc in range(ND):
                nc.tensor.matmul(ps_qk[:], w_qk[:, dc, :], xt[:, dc, :],
                                 start=(dc == 0), stop=(dc == ND - 1))
            ps_v = pwork.tile([H, CW], FP32, tag="pwork")
            for dc in range(ND):
                nc.tensor.matmul(ps_v[:], w_v[:, dc, :], xt[:, dc, :],
                                 start=(dc == 0), stop=(dc == ND - 1))

            qt = qkv.tile([H, CW], BF16, tag=f"qt{c}", name=f"qt{c}")
            kt = qkv.tile([H, CW], BF16, tag=f"kt{c}", name=f"kt{c}")
            vt = qkv.tile([H, CW], BF16, tag=f"vt{c}", name=f"vt{c}")
            nc.vector.tensor_copy(out=qt[:], in_=ps_qk[0:H, :])
            nc.vector.tensor_copy(out=kt[:], in_=ps_qk[H:P, :])
            nc.scalar.copy(out=vt[:], in_=ps_v[:])
            qt_chunks.append(qt)
            kt_chunks.append(kt)
            # V^T chunk -> V_aug k-tiles; xbar transpose on the ACT queue
            nc.sync.dma_start(out=v_aug[c][:, :, 0:H], in_=vt[:],
                              transpose=True)

            if c == 0:
                cast_transpose(2)

            # ---- attention: q-block [qlo, qlo+aw) handled in this chunk's
            #      body. Chunks 0-1 form one 1024-wide block (fewer, larger
            #      exp ops); chunks 2 and 3 run 512-wide so the tail chains
            #      are short and chunk-2's attention overlaps proj(3). ----
            if c % 2 == 0:
                qlo = aw = None
            else:
                qlo, aw = (c - 1) * CW, 2 * CW

            if aw is not None:
                ps_o = pout.tile([H + 1, aw], FP32, tag="pout",
                                 name=f"ps_o{c}")
                nki = (qlo + aw) // P     # k-tiles 0 .. nki-1 are valid

                def emit_s(ki, qlo=qlo, aw=aw, c=c):
                    c0, j0 = ki // KPC, ki % KPC
                    w = max(0, ki * P - qlo)
                    ps_s = pwork.tile([P, aw], FP32, tag="pwork",
                                      name=f"ps_s{c}_{ki}")
                    kts = kt_chunks[c0][:, j0 * P:(j0 + 1) * P]
                    for cq in range(qlo // CW, (qlo + aw) // CW):
                        lo = cq * CW - qlo       # block-local
                        hi = lo + CW
                        if hi <= w:
                            continue
                        s0 = max(w, lo)
                        nc.tensor.matmul(
                            ps_s[:, s0:hi], kts,
                            qt_chunks[cq][:, s0 - lo:CW],
                            start=True, stop=True)
                    pt = ptp.tile([P, aw], BF16, tag="pt", name=f"pt{c}_{ki}")
                    nc.scalar.activation(
                        out=pt[:, w:aw], in_=ps_s[:, w:aw],
                        func=mybir.ActivationFunctionType.Exp,
                        scale=0.125)
                    if ki * P >= qlo:
                        # causal diagonal: zero the strictly-lower triangle
                        nc.vector.tensor_mul(pt[:, w:w + P], pt[:, w:w + P],
                                             tri01[:])
                    return pt, w

                def emit_pv(ki, pt_w, qlo=qlo, aw=aw, nki_=None, c=c):
                    pt, w = pt_w
                    c0, j0 = ki // KPC, ki % KPC
                    for cq in range(qlo // CW, (qlo + aw) // CW):
                        lo = cq * CW - qlo
                        hi = lo + CW
                        if hi <= w:
                            continue
                        s0 = max(w, lo)
                        nc.tensor.matmul(
                            ps_o[:, s0:hi], v_aug[c0][:, j0, 0:H + 1],
                            pt[:, s0:hi],
                            start=(ki == 0), stop=(ki == nki_ - 1))

                pending = emit_s(0)
                for ki in range(nki):
                    nxt = emit_s(ki + 1) if ki + 1 < nki else None
                    if ki == 1 and out_stage:
                        out_stage.pop()()
                    emit_pv(ki, pending, nki_=nki)
                    pending = nxt

            if c == 1:
                cast_transpose(3)

            # ---- output stage for the q-block ----
            if aw is None:
                continue
            nq = aw // P
            ot = otp.tile([H + 1, aw], FP32, tag="ot", name=f"ot{c}")
            nc.vector.tensor_copy(out=ot[:], in_=ps_o[:])

            def do_out(c=c, ot=ot, qlo=qlo, aw=aw, nq=nq):
                for half in range(nq // KPC):
                    pst = pwork.tile([P, KPC, H + 1], FP32, tag="pwork",
                                     name=f"pst{c}_{half}")
                    for j in range(KPC):
                        jj = half * KPC + j
                        nc.tensor.transpose(pst[:, j, :],
                                            ot[:, jj * P:(jj + 1) * P],
                                            ident[0:H + 1, 0:H + 1])
                    rec = small.tile([P, KPC], FP32, tag="rec",
                                     name=f"rec{c}_{half}")
                    nc.vector.reciprocal(rec[:], pst[:, :, H])
                    ob = osb.tile([P, KPC, H], FP32, tag="ob",
                                  name=f"ob{c}_{half}")
                    for j in range(KPC):
                        nc.vector.tensor_scalar_mul(
                            ob[:, j, :], pst[:, j, 0:H], rec[:, j:j + 1])
                    stores.append(
                        (out_d.rearrange("(c a p) h -> c p a h",
                                         p=P, a=KPC)[qlo // CW + half], ob))

            if c < NC - 1:
                out_stage.append(do_out)
            else:
                do_out()

        # Stores issue on SP last so they never block the transpose queue;
        # data dependencies still gate each store.
        for dst, ob in stores:
            nc.gpsimd.dma_start(out=dst, in_=ob[:])

        if dbg:
            dpool = ctx.enter_context(tc.tile_pool(name="dbg", bufs=1))
            d1 = dpool.tile([P, ND, CW], FP32, name="d1")
            nc.vector.tensor_copy(out=d1[:], in_=xt_chunks[0][:])
            nc.sync.dma_start(out=dbg["xt0"], in_=d1[:])
            d2 = dpool.tile([P, KPC, H + 1], FP32, name="d2")
            nc.vector.tensor_copy(out=d2[:], in_=v_aug[0][:])
            nc.sync.dma_start(out=dbg["vaug0"], in_=d2[:])
            d3 = dpool.tile([H, CW], FP32, name="d3")
            nc.vector.tensor_copy(out=d3[:], in_=qt_chunks[0][:])
            nc.sync.dma_start(out=dbg["qt0"], in_=d3[:])
            d4 = dpool.tile([H, CW], FP32, name="d4")
            nc.vector.tensor_copy(out=d4[:], in_=kt_chunks[0][:])
            nc.sync.dma_start(out=dbg["kt0"], in_=d4[:])


def _run(inputs, trace=False, **kw):
    global _compiled
    if _compiled is None:
        _compiled = _build()
    nc = _compiled
    x = np.ascontiguousarray(inputs["x"], dtype=np.float32)
    wq = np.ascontiguousarray(inputs["Wq"], dtype=np.float32)
    wk = np.ascontiguousarray(inputs["Wk"], dtype=np.float32)
    wv = np.ascontiguousarray(inputs["Wv"], dtype=np.float32)
    in_maps = [
        {"x": np.ascontiguousarray(x[i]), "Wq": wq, "Wk": wk, "Wv": wv}
        for i in range(B)
    ]
    res = run_bass_kernel_spmd(nc, in_maps, core_ids=list(range(B)),
                               trace=trace, **kw)
    out = np.stack([res.results[i]["out"] for i in range(B)], axis=0)
    return out, res


def kernel(x, Wq, Wk, Wv):
    out, _ = _run({"x": x, "Wq": Wq, "Wk": Wk, "Wv": Wv})
    return out
